# revision 44
# baseline (speedup 1.0000x reference)
"""Cross-attention Trainium2 kernel (nn_CrossAttention_24575802868332).

Sharding: 8 cores; core c handles batch b = c//4 and query rows
r = (c%4)*1024 .. +1024.  Embarrassingly parallel, no collectives.
Host pre-transposes x/context slices and casts to bf16.

v2 design (vs baseline): q-outer two-chunk schedule with K/V resident
in SBUF (bf16), so the normalize + output projection of chunk 0 hides
under chunk 1's attention and only chunk 1's tail is exposed.  All
input DMAs are chunked and interleaved so the PE starts ~1us in.
Attention matmuls are bf16 (FWL weight loads); accumulation fp32.

Per-core schedule:
  P1: q^T = Wq^T x^T (kc-outer, 8 psum banks from the shared pools),
      interleaved with per-kc wq/xt DMA arrivals -> qt bf16 [128,4,1024]
  chunk c in {0,1} (q cols c*512..+512):
    for blk 0..7: 16 slabs (hp x mt):
      S^T pair -> psum [128,1024]; exp (scalar, 0.125 scale) -> bf16
      psl; O_aug pair accumulates into [65,512] psums (ones col of
      v_aug gives the softmax denominator l in row 64); per-hp flush
      to acc_c (fp32 SBUF)
    chunk0 sprinkles next block's K/V projection thunks (K^T via Wk
    lhsT; V via ctx lhsT) into the slab stream; K/V stay resident:
    kt [128,4,4096] bf16, vt [128,32,8,65] bf16.
    chunk1 sprinkles chunk0's P3 (1/l bcast + normalize + pair repack)
    and P4 (out = O/l @ Wo + bo) and output DMAs instead.
  tail: P3+P4 of chunk1 only (~12us exposed).
"""

import os
import sys

sys.path.insert(0, "/opt/trn_rl_repo")

from contextlib import ExitStack

import numpy as np

import concourse.bass as bass
import concourse.tile as tile
from concourse import bacc, mybir

F32 = mybir.dt.float32
F32R = mybir.dt.float32r
BF16 = mybir.dt.bfloat16
AF = mybir.ActivationFunctionType

# Problem constants (hardcoded per contract)
B, N, M = 2, 4096, 4096
DQ, DC, INNER = 1024, 768, 512
H, D = 8, 64
NCORES = 8
NQ = N * B // NCORES  # 1024 query rows per core
QC = 2  # q chunks of 512
QCW = NQ // QC  # 512
MBLK = 512  # m block size
NBLK = M // MBLK  # 8
HP = H // 2  # 4 head pairs
KQ = DQ // 128  # 8 k-chunks for q proj
KC = DC // 128  # 6 k-chunks for k/v proj
MS = MBLK // 128  # 4 m-subtiles per block


def build_nc():
    nc = bacc.Bacc(
        "TRN2",
        target_bir_lowering=False,
        debug=False,
        enable_asserts=False,
        num_devices=NCORES,
    )
    xT = nc.dram_tensor("xT", [DQ, NQ], BF16, kind="ExternalInput").ap()
    ctxT = nc.dram_tensor("ctxT", [DC, M], BF16, kind="ExternalInput").ap()
    wq = nc.dram_tensor("wq", [DQ, INNER], BF16, kind="ExternalInput").ap()
    wk = nc.dram_tensor("wk", [DC, INNER], BF16, kind="ExternalInput").ap()
    wv = nc.dram_tensor("wv", [DC, INNER], BF16, kind="ExternalInput").ap()
    wo = nc.dram_tensor("wo", [INNER, DQ], BF16, kind="ExternalInput").ap()
    bo = nc.dram_tensor("bo", [1, DQ], F32, kind="ExternalInput").ap()
    ones_d = nc.dram_tensor("ones_d", [1, 128], BF16, kind="ExternalInput").ap()
    out = nc.dram_tensor("out", [NQ, DQ], F32, kind="ExternalOutput").ap()

    with tile.TileContext(nc) as tc:
        _emit(nc, tc, xT, ctxT, wq, wk, wv, wo, bo, ones_d, out)
    nc.compile()
    return nc


def _emit(nc, tc, xT, ctxT, wq, wk, wv, wo, bo, ones_d, out):
    with ExitStack() as ctx:
        consts = ctx.enter_context(tc.tile_pool(name="consts", bufs=1))
        persist = ctx.enter_context(tc.tile_pool(name="persist", bufs=1))
        ctx_pool = ctx.enter_context(tc.tile_pool(name="ctx", bufs=2))
        p_pool = ctx.enter_context(tc.tile_pool(name="p", bufs=3))
        out_pool = ctx.enter_context(tc.tile_pool(name="outp", bufs=2))
        norm = ctx.enter_context(tc.tile_pool(name="norm", bufs=1))
        s_ps = ctx.enter_context(tc.tile_pool(name="sps", bufs=2, space="PSUM"))
        o_ps = ctx.enter_context(tc.tile_pool(name="ops", bufs=4, space="PSUM"))

        # ---- interleaved input DMAs: wq/xt chunks first so P1 starts early
        wq_sb = consts.tile([128, KQ, INNER], BF16, tag="wq")
        xt_sb = consts.tile([128, KQ, NQ], BF16, tag="xt")
        wqr = wq.rearrange("(k p) n -> p k n", p=128)
        xTr = xT.rearrange("(k p) q -> p k q", p=128)
        for kc in range(KQ):
            nc.sync.dma_start(out=wq_sb[:, kc, :], in_=wqr[:, kc, :])
            nc.sync.dma_start(out=xt_sb[:, kc, :], in_=xTr[:, kc, :])
        wk_sb = consts.tile([128, KC, INNER], BF16, tag="wk")
        nc.sync.dma_start(out=wk_sb, in_=wk.rearrange("(k p) n -> p k n", p=128))
        wv_sb = consts.tile([128, KC, INNER], BF16, tag="wv")
        nc.sync.dma_start(out=wv_sb, in_=wv.rearrange("(k p) n -> p k n", p=128))
        ctxTr = ctxT.rearrange("(k p) m -> p k m", p=128)
        wo_sb = consts.tile([128, INNER // 128, DQ], BF16, tag="wo")
        nc.sync.dma_start(out=wo_sb, in_=wo.rearrange("(k p) n -> p k n", p=128))
        ones_row = consts.tile([1, 128], BF16, tag="ones_row")
        nc.sync.dma_start(out=ones_row, in_=ones_d)
        # bias broadcast to all 128 partitions
        bias_bc = consts.tile([128, DQ], F32, tag="bias_bc")
        bias_ap = bass.AP(tensor=bo.tensor, offset=0, ap=[[0, 128], [1, DQ]])
        nc.gpsimd.dma_start(out=bias_bc, in_=bias_ap)

        # persistent attention state
        qt_sb = persist.tile([128, HP, NQ], BF16, tag="qt")
        kt_all = persist.tile([128, HP, M], BF16, tag="kt")
        vt_all = persist.tile([128, NBLK * MS, H, 65], BF16, tag="vt")
        acc = [
            persist.tile([65, H, QCW], F32, tag=f"acc{c}", name=f"acc{c}")
            for c in range(QC)
        ]
        # single-buffered (ring) across chunks: chunk1's writers naturally
        # wait for chunk0's last readers, which finish first
        ko = [
            norm.tile([128, HP, QCW], BF16, tag="ko", name=f"ko{c}")
            for c in range(QC)
        ]
        nodd = [
            norm.tile([64, HP, QCW], BF16, tag="nodd", name=f"nodd{c}")
            for c in range(QC)
        ]
        # v_aug ones column (written once; later evacs only touch cols 0:64)
        nc.vector.memset(vt_all[:, :, :, 64:65], 1.0)

        # ---- P1: q^T projection in two passes of two head pairs each,
        # using only the s_ps ring (so block 0's K/V projections can
        # interleave into pass 2 through the free o_ps ring) ----
        def p1_pass(its, pend):
            qp = {}
            per = (len(pend) + KQ - 1) // KQ if pend else 0
            for kc in range(KQ):
                for it in its:
                    if kc == 0:
                        big = s_ps.tile(
                            [128, 1024], F32, tag="sps", name=f"qp{it}"
                        )
                        qp[it] = big
                    for qh in range(2):
                        nc.tensor.matmul(
                            qp[it][:, qh * 512 : (qh + 1) * 512],
                            wq_sb[:, kc, it * 128 : (it + 1) * 128],
                            xt_sb[:, kc, qh * 512 : (qh + 1) * 512],
                            start=(kc == 0),
                            stop=(kc == KQ - 1),
                        )
                for _ in range(per):
                    if pend:
                        pend.pop(0)()
            for it in its:
                nc.vector.tensor_copy(qt_sb[:, it, :], qp[it])
            return pend

        # ---- K/V projection thunks for one m-block ----
        def make_proj_thunks(blk):
            st = {}
            th = []

            def t_dma(blk=blk):
                cx = ctx_pool.tile([128, KC, MBLK], BF16, tag="cx", name=f"cx{blk}")
                m0 = blk * MBLK
                nc.sync.dma_start(out=cx, in_=ctxTr[:, :, m0 : m0 + MBLK])
                st["cx"] = cx

            th.append(t_dma)
            for it in range(HP):
                for kc in range(KC):
                    def t_kmm(it=it, kc=kc, blk=blk):
                        if kc == 0:
                            st[f"kp{it}"] = o_ps.tile(
                                [128, 512], F32, tag="ops", name=f"kp{blk}_{it}"
                            )
                        nc.tensor.matmul(
                            st[f"kp{it}"],
                            wk_sb[:, kc, it * 128 : (it + 1) * 128],
                            st["cx"][:, kc, :],
                            start=(kc == 0),
                            stop=(kc == KC - 1),
                        )
                    th.append(t_kmm)

                def t_kev(it=it, blk=blk):
                    nc.vector.tensor_copy(
                        kt_all[:, it, blk * MBLK : (blk + 1) * MBLK], st[f"kp{it}"]
                    )

                th.append(t_kev)
            for ms in range(MS):
                for kc in range(KC):
                    def t_vmm(ms=ms, kc=kc, blk=blk):
                        if kc == 0:
                            st[f"vp{ms}"] = o_ps.tile(
                                [128, 512], F32, tag="ops", name=f"vp{blk}_{ms}"
                            )
                        nc.tensor.matmul(
                            st[f"vp{ms}"],
                            st["cx"][:, kc, ms * 128 : (ms + 1) * 128],
                            wv_sb[:, kc, :],
                            start=(kc == 0),
                            stop=(kc == KC - 1),
                        )
                    th.append(t_vmm)

                def t_vev(ms=ms, blk=blk):
                    nc.vector.tensor_copy(
                        vt_all[:, blk * MS + ms, :, 0:64],
                        st[f"vp{ms}"][:].rearrange("p (h d) -> p h d", h=H),
                    )

                th.append(t_vev)
            return th

        # ---- P3: normalize one head pair of a chunk accumulator and
        # repack it into ko's pair layout (injected right after that
        # pair's final flush so it overlaps the last block's slabs) ----
        _p3_state = {}
        def make_p3_hp_thunks(c, hp, part=None):
            """part=None: full chain; 'head': recip DMAs only;
            'tail': bcast/mul/repack (inject a flush later so the head's
            DMA latency never stalls the PE queue)."""
            st = _p3_state.setdefault((c, hp), {})
            th = []

            def t_rd(c=c, hp=hp):
                r2 = norm.tile([2, QCW], F32R, tag="r2", name=f"r2_{c}_{hp}")
                # gpsimd DMA queue: keeps these small transfers from
                # queuing behind 512KB output DMAs on the sync queue
                nc.gpsimd.dma_start(
                    out=r2,
                    in_=acc[c][64:65, 2 * hp : 2 * hp + 2, :].bitcast(F32R),
                )
                st["r2"] = r2

            def t_rc(c=c, hp=hp):
                rb2 = norm.tile([2, QCW], BF16, tag="rb2", name=f"rb2_{c}_{hp}")
                with nc.allow_low_precision(reason="1/l in bf16 is fine"):
                    nc.vector.reciprocal(rb2[:], st["r2"][:])
                st["rb2"] = rb2

            def t_rb(c=c, hp=hp):
                rr = norm.tile([1, 2, QCW], BF16, tag="rr", name=f"rr_{c}_{hp}")
                nc.gpsimd.dma_start(out=rr, in_=st["rb2"][:, :])
                st["rr"] = rr

            if part != "tail":
                th += [t_rd, t_rc, t_rb]
            if part == "head":
                return th
            for par in range(2):
                def t_bcast(par=par, c=c, hp=hp):
                    bp = o_ps.tile(
                        [64, 512], F32, tag="ops", name=f"bp{c}_{hp}_{par}"
                    )
                    nc.tensor.matmul(
                        bp,
                        ones_row[0:1, 0:64],
                        st["rr"][:, par, :],
                        start=True,
                        stop=True,
                    )
                    st[f"bp{par}"] = bp

                def t_mul(par=par, c=c, hp=hp):
                    dst = ko[c][0:64, hp, :] if par == 0 else nodd[c][:, hp, :]
                    nc.vector.tensor_mul(
                        dst, acc[c][0:64, 2 * hp + par, :], st[f"bp{par}"]
                    )

                th += [t_bcast, t_mul]

            def t_rp(hp=hp, c=c):
                nc.gpsimd.dma_start(
                    out=ko[c][64:128, hp, :], in_=nodd[c][:, hp, :]
                )

            th.append(t_rp)
            return th

        # ---- P4: out = (O/l) @ Wo + bo for one chunk ----
        def make_p4_thunks(c):
            st = {}
            th = []
            for qi in range(QCW // 128):
                def t_alloc(qi=qi, c=c):
                    st[f"ob{qi}"] = out_pool.tile(
                        [128, DQ], F32, tag="ob", name=f"ob{c}_{qi}"
                    )

                th.append(t_alloc)
                for nck in range(DQ // 512):
                    for hp in range(HP):
                        def t_mm(qi=qi, nck=nck, hp=hp, c=c):
                            if hp == 0:
                                st[f"pp{qi}_{nck}"] = o_ps.tile(
                                    [128, 512], F32, tag="ops",
                                    name=f"pp{c}_{qi}_{nck}",
                                )
                            nc.tensor.matmul(
                                st[f"pp{qi}_{nck}"],
                                ko[c][:, hp, qi * 128 : (qi + 1) * 128],
                                wo_sb[:, hp, nck * 512 : (nck + 1) * 512],
                                start=(hp == 0),
                                stop=(hp == HP - 1),
                            )
                        th.append(t_mm)

                    def t_ev(qi=qi, nck=nck):
                        nc.vector.tensor_add(
                            st[f"ob{qi}"][:, nck * 512 : (nck + 1) * 512],
                            st[f"pp{qi}_{nck}"],
                            bias_bc[:, nck * 512 : (nck + 1) * 512],
                        )

                    th.append(t_ev)

                def t_dma(qi=qi, c=c):
                    r0 = c * QCW + qi * 128
                    nc.sync.dma_start(
                        out=out[r0 : r0 + 128, :], in_=st[f"ob{qi}"]
                    )

                th.append(t_dma)
            return th

        # ---- attention slab stream: one (chunk, block) group ----
        def run_block(c, blk, pend, horizon):
            """16 slabs (hp x mt); pops background thunks from pend,
            paced so the backlog drains over `horizon` slabs (which may
            span into following groups).  On the chunk's last block,
            injects that head pair's P3 chain right after its flush.
            Returns the remaining pend (carried into the next group).
            """
            q0 = c * QCW
            if True:
                for hp in range(HP):
                    per = (
                        (len(pend) + max(1, horizon) - 1) // max(1, horizon)
                        if pend
                        else 0
                    )
                    ops_e = o_ps.tile(
                        [65, 512], F32, tag="ops", name=f"oe{c}_{blk}_{hp}"
                    )
                    ops_o = o_ps.tile(
                        [65, 512], F32, tag="ops", name=f"oo{c}_{blk}_{hp}"
                    )
                    o_emits = []
                    for mt in range(MS):
                        sl = s_ps.tile(
                            [128, 1024], F32, tag="sps",
                            name=f"sl{c}_{blk}_{hp}_{mt}",
                        )
                        mofs = blk * MBLK + mt * 128
                        nc.tensor.matmul(
                            sl[:, 0:512],
                            kt_all[0:64, hp, mofs : mofs + 128],
                            qt_sb[0:64, hp, q0 : q0 + QCW],
                            start=True,
                            stop=True,
                        )
                        nc.tensor.matmul(
                            sl[:, 512:1024],
                            kt_all[64:128, hp, mofs : mofs + 128],
                            qt_sb[64:128, hp, q0 : q0 + QCW],
                            start=True,
                            stop=True,
                        )
                        psl = p_pool.tile(
                            [128, 1024], BF16, tag="p",
                            name=f"psl{c}_{blk}_{hp}_{mt}",
                        )
                        nc.scalar.activation(psl, sl, AF.Exp, scale=0.125)

                        def o_pair(mt=mt, psl=psl, ops_e=ops_e, ops_o=ops_o,
                                   blk=blk, hp=hp):
                            bms = blk * MS + mt
                            nc.tensor.matmul(
                                ops_e,
                                vt_all[:, bms, 2 * hp, :],
                                psl[:, 0:512],
                                start=(mt == 0),
                                stop=(mt == MS - 1),
                            )
                            nc.tensor.matmul(
                                ops_o,
                                vt_all[:, bms, 2 * hp + 1, :],
                                psl[:, 512:1024],
                                start=(mt == 0),
                                stop=(mt == MS - 1),
                            )

                        o_emits.append(o_pair)
                        # pops go between S and the lagged O pair: the PE
                        # chews background work while the previous slab's
                        # exp finishes, instead of stalling on psl
                        for _ in range(per):
                            if pend:
                                pend.pop(0)()
                        horizon -= 1
                        if mt >= 1:
                            o_emits.pop(0)()
                    while o_emits:
                        o_emits.pop(0)()
                    # flush O_aug psums into the chunk accumulator
                    if blk == 0:
                        nc.vector.tensor_copy(acc[c][:, 2 * hp, :], ops_e)
                        nc.vector.tensor_copy(acc[c][:, 2 * hp + 1, :], ops_o)
                    else:
                        nc.vector.tensor_add(
                            acc[c][:, 2 * hp, :], acc[c][:, 2 * hp, :], ops_e
                        )
                        nc.vector.tensor_add(
                            acc[c][:, 2 * hp + 1, :], acc[c][:, 2 * hp + 1, :],
                            ops_o,
                        )
                    if blk == NBLK - 1:
                        pend = pend + make_p3_hp_thunks(c, hp, part="head")
                        if hp > 0:
                            pend = pend + make_p3_hp_thunks(
                                c, hp - 1, part="tail"
                            )
            if blk == NBLK - 1:
                pend = pend + make_p3_hp_thunks(c, HP - 1, part="tail")
            return pend, horizon

        p1_pass((0, 1), [])
        for t in p1_pass((2, 3), make_proj_thunks(0)):
            t()
        # interleave (chunk, block) groups so block b+1's projection
        # spreads over BOTH groups that process block b (32 slabs),
        # keeping every slab PE-bound instead of exp-bound; (0,7) comes
        # before (1,6)/(1,7) so chunk 0's P3/P4 hide under them.
        groups = [(0, 0)]
        for b in range(6):
            groups += [(1, b), (0, b + 1)]
        groups += [(0, 7), (1, 6), (1, 7)]
        pend, horizon = [], 16
        for c, blk in groups:
            if c == 0 and blk + 1 < NBLK:
                pend = pend + make_proj_thunks(blk + 1)
                horizon = 16 if blk == 6 else 32
            if (c, blk) == (1, 6):
                pend = pend + make_p4_thunks(0)
                horizon = 32
            horizon = max(horizon, 16) if pend else 16
            pend, horizon = run_block(c, blk, pend, horizon)
        # tail: chunk 1's last P3 chain + P4(1).  Per-group thunks are
        # [alloc, mm hp0..3, ev] x2 + dma (12 each).  Emit the recip
        # DMAs, then two groups' alloc+hp0-2 matmuls (their psums plus
        # the two bcast psums exactly fill the o_ps ring) so the PE
        # stays busy while the chain flies, then the chain, then the
        # gated hp3 matmuls and the remaining groups.
        p4 = make_p4_thunks(1)
        g0, g1, rest = p4[0:12], p4[12:24], p4[24:]
        # pend's last 5 thunks are always hp3's P3 tail (appended last,
        # FIFO pops take from the front); everything before it must run
        # before P4's hp0-2 matmuls (ko writers).  Hoist two groups'
        # alloc+hp0-2 matmuls over the hp3 tail so the PE stays busy
        # while its bcast waits on the recip-chain DMAs.
        tail = (
            pend[:-5]
            + g0[0:4] + g1[0:4]          # allocs + mm hp0-2 of nck 0
            + pend[-5:]                   # hp3 bcast/mul/repack
            + g0[4:] + g1[4:]            # mm hp3, ev, second nck, dma
            + rest
        )
        for t in tail:
            t()


_NC_CACHE = None


def _get_nc():
    global _NC_CACHE
    if _NC_CACHE is None:
        _NC_CACHE = build_nc()
    return _NC_CACHE


def shard_inputs(x, context, Wq, Wk, Wv, Wo, bo):
    import ml_dtypes

    bf16 = ml_dtypes.bfloat16
    ones = np.ones((1, 128), np.float32).astype(bf16)
    bo2 = np.ascontiguousarray(np.asarray(bo, np.float32).reshape(1, DQ))
    Wq = np.ascontiguousarray(np.asarray(Wq, np.float32).astype(bf16))
    Wk = np.ascontiguousarray(np.asarray(Wk, np.float32).astype(bf16))
    Wv = np.ascontiguousarray(np.asarray(Wv, np.float32).astype(bf16))
    Wo = np.ascontiguousarray(np.asarray(Wo, np.float32).astype(bf16))
    maps = []
    for c in range(NCORES):
        b = c // 4
        r0 = (c % 4) * NQ
        maps.append(
            {
                "xT": np.ascontiguousarray(x[b, r0 : r0 + NQ, :].T.astype(bf16)),
                "ctxT": np.ascontiguousarray(context[b].T.astype(bf16)),
                "wq": Wq,
                "wk": Wk,
                "wv": Wv,
                "wo": Wo,
                "bo": bo2,
                "ones_d": ones,
            }
        )
    return maps


def kernel(x, context, Wq, Wk, Wv, Wo, bo):
    from concourse.bass_utils import run_bass_kernel_spmd

    x = np.asarray(x, np.float32)
    context = np.asarray(context, np.float32)
    maps = shard_inputs(x, context, Wq, Wk, Wv, Wo, bo)
    nc = _get_nc()
    trace = os.environ.get("KERNEL_TRACE", "0") == "1"
    res = run_bass_kernel_spmd(
        nc, maps, core_ids=list(range(NCORES)), trace=trace
    )
    full = np.empty((B, N, DQ), np.float32)
    for c in range(NCORES):
        b = c // 4
        r0 = (c % 4) * NQ
        full[b, r0 : r0 + NQ, :] = res.results[c]["out"]
    if trace:
        kernel.last_exec_time_ns = res.exec_time_ns
    return full


# revision 45
# speedup vs baseline: 1.0108x; 1.0108x over previous
"""Cross-attention Trainium2 kernel (nn_CrossAttention_24575802868332).

Sharding: 8 cores; core c handles batch b = c//4 and query rows
r = (c%4)*1024 .. +1024.  Embarrassingly parallel, no collectives.
Host pre-transposes x/context slices and casts to bf16.

v2 design (vs baseline): q-outer two-chunk schedule with K/V resident
in SBUF (bf16), so the normalize + output projection of chunk 0 hides
under chunk 1's attention and only chunk 1's tail is exposed.  All
input DMAs are chunked and interleaved so the PE starts ~1us in.
Attention matmuls are bf16 (FWL weight loads); accumulation fp32.

Per-core schedule:
  P1: q^T = Wq^T x^T (kc-outer, 8 psum banks from the shared pools),
      interleaved with per-kc wq/xt DMA arrivals -> qt bf16 [128,4,1024]
  chunk c in {0,1} (q cols c*512..+512):
    for blk 0..7: 16 slabs (hp x mt):
      S^T pair -> psum [128,1024]; exp (scalar, 0.125 scale) -> bf16
      psl; O_aug pair accumulates into [65,512] psums (ones col of
      v_aug gives the softmax denominator l in row 64); per-hp flush
      to acc_c (fp32 SBUF)
    chunk0 sprinkles next block's K/V projection thunks (K^T via Wk
    lhsT; V via ctx lhsT) into the slab stream; K/V stay resident:
    kt [128,4,4096] bf16, vt [128,32,8,65] bf16.
    chunk1 sprinkles chunk0's P3 (1/l bcast + normalize + pair repack)
    and P4 (out = O/l @ Wo + bo) and output DMAs instead.
  tail: P3+P4 of chunk1 only (~12us exposed).
"""

import os
import sys

sys.path.insert(0, "/opt/trn_rl_repo")

from contextlib import ExitStack

import numpy as np

import concourse.bass as bass
import concourse.tile as tile
from concourse import bacc, mybir

F32 = mybir.dt.float32
F32R = mybir.dt.float32r
BF16 = mybir.dt.bfloat16
AF = mybir.ActivationFunctionType

# Problem constants (hardcoded per contract)
B, N, M = 2, 4096, 4096
DQ, DC, INNER = 1024, 768, 512
H, D = 8, 64
NCORES = 8
NQ = N * B // NCORES  # 1024 query rows per core
QC = 2  # q chunks of 512
QCW = NQ // QC  # 512
MBLK = 512  # m block size
NBLK = M // MBLK  # 8
HP = H // 2  # 4 head pairs
KQ = DQ // 128  # 8 k-chunks for q proj
KC = DC // 128  # 6 k-chunks for k/v proj
MS = MBLK // 128  # 4 m-subtiles per block


def build_nc():
    nc = bacc.Bacc(
        "TRN2",
        target_bir_lowering=False,
        debug=False,
        enable_asserts=False,
        num_devices=NCORES,
    )
    xT = nc.dram_tensor("xT", [DQ, NQ], BF16, kind="ExternalInput").ap()
    ctxT = nc.dram_tensor("ctxT", [DC, M], BF16, kind="ExternalInput").ap()
    wq = nc.dram_tensor("wq", [DQ, INNER], BF16, kind="ExternalInput").ap()
    wk = nc.dram_tensor("wk", [DC, INNER], BF16, kind="ExternalInput").ap()
    wv = nc.dram_tensor("wv", [DC, INNER], BF16, kind="ExternalInput").ap()
    wo = nc.dram_tensor("wo", [INNER, DQ], BF16, kind="ExternalInput").ap()
    bo = nc.dram_tensor("bo", [1, DQ], F32, kind="ExternalInput").ap()
    ones_d = nc.dram_tensor("ones_d", [1, 128], BF16, kind="ExternalInput").ap()
    out = nc.dram_tensor("out", [NQ, DQ], F32, kind="ExternalOutput").ap()

    with tile.TileContext(nc) as tc:
        _emit(nc, tc, xT, ctxT, wq, wk, wv, wo, bo, ones_d, out)
    nc.compile()
    return nc


def _emit(nc, tc, xT, ctxT, wq, wk, wv, wo, bo, ones_d, out):
    with ExitStack() as ctx:
        consts = ctx.enter_context(tc.tile_pool(name="consts", bufs=1))
        persist = ctx.enter_context(tc.tile_pool(name="persist", bufs=1))
        ctx_pool = ctx.enter_context(tc.tile_pool(name="ctx", bufs=2))
        p_pool = ctx.enter_context(tc.tile_pool(name="p", bufs=3))
        out_pool = ctx.enter_context(tc.tile_pool(name="outp", bufs=2))
        norm = ctx.enter_context(tc.tile_pool(name="norm", bufs=1))
        s_ps = ctx.enter_context(tc.tile_pool(name="sps", bufs=2, space="PSUM"))
        o_ps = ctx.enter_context(tc.tile_pool(name="ops", bufs=4, space="PSUM"))

        # ---- interleaved input DMAs: wq/xt chunks first so P1 starts early
        wq_sb = consts.tile([128, KQ, INNER], BF16, tag="wq")
        xt_sb = consts.tile([128, KQ, NQ], BF16, tag="xt")
        wqr = wq.rearrange("(k p) n -> p k n", p=128)
        xTr = xT.rearrange("(k p) q -> p k q", p=128)
        for kc in range(KQ):
            nc.sync.dma_start(out=wq_sb[:, kc, :], in_=wqr[:, kc, :])
            nc.sync.dma_start(out=xt_sb[:, kc, :], in_=xTr[:, kc, :])
        wk_sb = consts.tile([128, KC, INNER], BF16, tag="wk")
        nc.sync.dma_start(out=wk_sb, in_=wk.rearrange("(k p) n -> p k n", p=128))
        wv_sb = consts.tile([128, KC, INNER], BF16, tag="wv")
        nc.sync.dma_start(out=wv_sb, in_=wv.rearrange("(k p) n -> p k n", p=128))
        ctxTr = ctxT.rearrange("(k p) m -> p k m", p=128)
        wo_sb = consts.tile([128, INNER // 128, DQ], BF16, tag="wo")
        nc.sync.dma_start(out=wo_sb, in_=wo.rearrange("(k p) n -> p k n", p=128))
        ones_row = consts.tile([1, 128], BF16, tag="ones_row")
        nc.sync.dma_start(out=ones_row, in_=ones_d)
        # bias broadcast to all 128 partitions
        bias_bc = consts.tile([128, DQ], F32, tag="bias_bc")
        bias_ap = bass.AP(tensor=bo.tensor, offset=0, ap=[[0, 128], [1, DQ]])
        nc.gpsimd.dma_start(out=bias_bc, in_=bias_ap)

        # persistent attention state
        qt_sb = persist.tile([128, HP, NQ], BF16, tag="qt")
        kt_all = persist.tile([128, HP, M], BF16, tag="kt")
        vt_all = persist.tile([128, NBLK * MS, H, 65], BF16, tag="vt")
        acc = [
            persist.tile([65, H, QCW], F32, tag=f"acc{c}", name=f"acc{c}")
            for c in range(QC)
        ]
        # single-buffered (ring) across chunks: chunk1's writers naturally
        # wait for chunk0's last readers, which finish first
        ko = [
            norm.tile([128, HP, QCW], BF16, tag="ko", name=f"ko{c}")
            for c in range(QC)
        ]
        nodd = [
            norm.tile([64, HP, QCW], BF16, tag="nodd", name=f"nodd{c}")
            for c in range(QC)
        ]
        # v_aug ones column (written once; later evacs only touch cols 0:64)
        nc.vector.memset(vt_all[:, :, :, 64:65], 1.0)

        # ---- P1: q^T projection in two passes of two head pairs each,
        # using only the s_ps ring (so block 0's K/V projections can
        # interleave into pass 2 through the free o_ps ring) ----
        def p1_pass(its, pend):
            qp = {}
            per = (len(pend) + KQ - 1) // KQ if pend else 0
            for kc in range(KQ):
                for it in its:
                    if kc == 0:
                        big = s_ps.tile(
                            [128, 1024], F32, tag="sps", name=f"qp{it}"
                        )
                        qp[it] = big
                    for qh in range(2):
                        nc.tensor.matmul(
                            qp[it][:, qh * 512 : (qh + 1) * 512],
                            wq_sb[:, kc, it * 128 : (it + 1) * 128],
                            xt_sb[:, kc, qh * 512 : (qh + 1) * 512],
                            start=(kc == 0),
                            stop=(kc == KQ - 1),
                        )
                for _ in range(per):
                    if pend:
                        pend.pop(0)()
            for it in its:
                nc.vector.tensor_copy(qt_sb[:, it, :], qp[it])
            return pend

        # ---- K/V projection thunks for one m-block ----
        def make_proj_thunks(blk):
            st = {}
            th = []

            def t_dma(blk=blk):
                cx = ctx_pool.tile([128, KC, MBLK], BF16, tag="cx", name=f"cx{blk}")
                m0 = blk * MBLK
                nc.sync.dma_start(out=cx, in_=ctxTr[:, :, m0 : m0 + MBLK])
                st["cx"] = cx

            th.append(t_dma)
            for it in range(HP):
                for kc in range(KC):
                    def t_kmm(it=it, kc=kc, blk=blk):
                        if kc == 0:
                            st[f"kp{it}"] = o_ps.tile(
                                [128, 512], F32, tag="ops", name=f"kp{blk}_{it}"
                            )
                        nc.tensor.matmul(
                            st[f"kp{it}"],
                            wk_sb[:, kc, it * 128 : (it + 1) * 128],
                            st["cx"][:, kc, :],
                            start=(kc == 0),
                            stop=(kc == KC - 1),
                        )
                    th.append(t_kmm)

                def t_kev(it=it, blk=blk):
                    nc.vector.tensor_copy(
                        kt_all[:, it, blk * MBLK : (blk + 1) * MBLK], st[f"kp{it}"]
                    )

                th.append(t_kev)
            for ms in range(MS):
                for kc in range(KC):
                    def t_vmm(ms=ms, kc=kc, blk=blk):
                        if kc == 0:
                            st[f"vp{ms}"] = o_ps.tile(
                                [128, 512], F32, tag="ops", name=f"vp{blk}_{ms}"
                            )
                        nc.tensor.matmul(
                            st[f"vp{ms}"],
                            st["cx"][:, kc, ms * 128 : (ms + 1) * 128],
                            wv_sb[:, kc, :],
                            start=(kc == 0),
                            stop=(kc == KC - 1),
                        )
                    th.append(t_vmm)

                def t_vev(ms=ms, blk=blk):
                    nc.vector.tensor_copy(
                        vt_all[:, blk * MS + ms, :, 0:64],
                        st[f"vp{ms}"][:].rearrange("p (h d) -> p h d", h=H),
                    )

                th.append(t_vev)
            return th

        # ---- P3: normalize one head pair of a chunk accumulator and
        # repack it into ko's pair layout (injected right after that
        # pair's final flush so it overlaps the last block's slabs) ----
        _p3_state = {}
        def make_p3_hp_thunks(c, hp, part=None):
            """part=None: full chain; 'head': recip DMAs only;
            'tail': bcast/mul/repack (inject a flush later so the head's
            DMA latency never stalls the PE queue)."""
            st = _p3_state.setdefault((c, hp), {})
            th = []

            def t_rd(c=c, hp=hp):
                r2 = norm.tile([2, QCW], F32R, tag="r2", name=f"r2_{c}_{hp}")
                nc.sync.dma_start(
                    out=r2,
                    in_=acc[c][64:65, 2 * hp : 2 * hp + 2, :].bitcast(F32R),
                )
                st["r2"] = r2

            def t_rc(c=c, hp=hp):
                rb2 = norm.tile([2, QCW], BF16, tag="rb2", name=f"rb2_{c}_{hp}")
                with nc.allow_low_precision(reason="1/l in bf16 is fine"):
                    nc.vector.reciprocal(rb2[:], st["r2"][:])
                st["rb2"] = rb2

            def t_rb(c=c, hp=hp):
                rr = norm.tile([1, 2, QCW], BF16, tag="rr", name=f"rr_{c}_{hp}")
                nc.sync.dma_start(out=rr, in_=st["rb2"][:, :])
                st["rr"] = rr

            if part != "tail":
                th += [t_rd, t_rc, t_rb]
            if part == "head":
                return th
            for par in range(2):
                def t_bcast(par=par, c=c, hp=hp):
                    bp = o_ps.tile(
                        [64, 512], F32, tag="ops", name=f"bp{c}_{hp}_{par}"
                    )
                    nc.tensor.matmul(
                        bp,
                        ones_row[0:1, 0:64],
                        st["rr"][:, par, :],
                        start=True,
                        stop=True,
                    )
                    st[f"bp{par}"] = bp

                def t_mul(par=par, c=c, hp=hp):
                    dst = ko[c][0:64, hp, :] if par == 0 else nodd[c][:, hp, :]
                    nc.vector.tensor_mul(
                        dst, acc[c][0:64, 2 * hp + par, :], st[f"bp{par}"]
                    )

                th += [t_bcast, t_mul]

            def t_rp(hp=hp, c=c):
                nc.sync.dma_start(
                    out=ko[c][64:128, hp, :], in_=nodd[c][:, hp, :]
                )

            th.append(t_rp)
            return th

        # ---- P4: out = (O/l) @ Wo + bo for one chunk ----
        def make_p4_thunks(c):
            st = {}
            th = []
            for qi in range(QCW // 128):
                def t_alloc(qi=qi, c=c):
                    st[f"ob{qi}"] = out_pool.tile(
                        [128, DQ], F32, tag="ob", name=f"ob{c}_{qi}"
                    )

                th.append(t_alloc)
                for nck in range(DQ // 512):
                    for hp in range(HP):
                        def t_mm(qi=qi, nck=nck, hp=hp, c=c):
                            if hp == 0:
                                st[f"pp{qi}_{nck}"] = o_ps.tile(
                                    [128, 512], F32, tag="ops",
                                    name=f"pp{c}_{qi}_{nck}",
                                )
                            nc.tensor.matmul(
                                st[f"pp{qi}_{nck}"],
                                ko[c][:, hp, qi * 128 : (qi + 1) * 128],
                                wo_sb[:, hp, nck * 512 : (nck + 1) * 512],
                                start=(hp == 0),
                                stop=(hp == HP - 1),
                            )
                        th.append(t_mm)

                    def t_ev(qi=qi, nck=nck):
                        nc.vector.tensor_add(
                            st[f"ob{qi}"][:, nck * 512 : (nck + 1) * 512],
                            st[f"pp{qi}_{nck}"],
                            bias_bc[:, nck * 512 : (nck + 1) * 512],
                        )

                    th.append(t_ev)

                def t_dma(qi=qi, c=c):
                    r0 = c * QCW + qi * 128
                    nc.sync.dma_start(
                        out=out[r0 : r0 + 128, :], in_=st[f"ob{qi}"]
                    )

                th.append(t_dma)
            return th

        # ---- attention slab stream: one (chunk, block) group ----
        def run_block(c, blk, pend, horizon):
            """16 slabs (hp x mt); pops background thunks from pend,
            paced so the backlog drains over `horizon` slabs (which may
            span into following groups).  On the chunk's last block,
            injects that head pair's P3 chain right after its flush.
            Returns the remaining pend (carried into the next group).
            """
            q0 = c * QCW
            if True:
                for hp in range(HP):
                    per = (
                        (len(pend) + max(1, horizon) - 1) // max(1, horizon)
                        if pend
                        else 0
                    )
                    ops_e = o_ps.tile(
                        [65, 512], F32, tag="ops", name=f"oe{c}_{blk}_{hp}"
                    )
                    ops_o = o_ps.tile(
                        [65, 512], F32, tag="ops", name=f"oo{c}_{blk}_{hp}"
                    )
                    o_emits = []
                    for mt in range(MS):
                        sl = s_ps.tile(
                            [128, 1024], F32, tag="sps",
                            name=f"sl{c}_{blk}_{hp}_{mt}",
                        )
                        mofs = blk * MBLK + mt * 128
                        nc.tensor.matmul(
                            sl[:, 0:512],
                            kt_all[0:64, hp, mofs : mofs + 128],
                            qt_sb[0:64, hp, q0 : q0 + QCW],
                            start=True,
                            stop=True,
                        )
                        nc.tensor.matmul(
                            sl[:, 512:1024],
                            kt_all[64:128, hp, mofs : mofs + 128],
                            qt_sb[64:128, hp, q0 : q0 + QCW],
                            start=True,
                            stop=True,
                        )
                        psl = p_pool.tile(
                            [128, 1024], BF16, tag="p",
                            name=f"psl{c}_{blk}_{hp}_{mt}",
                        )
                        nc.scalar.activation(psl, sl, AF.Exp, scale=0.125)

                        def o_pair(mt=mt, psl=psl, ops_e=ops_e, ops_o=ops_o,
                                   blk=blk, hp=hp):
                            bms = blk * MS + mt
                            nc.tensor.matmul(
                                ops_e,
                                vt_all[:, bms, 2 * hp, :],
                                psl[:, 0:512],
                                start=(mt == 0),
                                stop=(mt == MS - 1),
                            )
                            nc.tensor.matmul(
                                ops_o,
                                vt_all[:, bms, 2 * hp + 1, :],
                                psl[:, 512:1024],
                                start=(mt == 0),
                                stop=(mt == MS - 1),
                            )

                        o_emits.append(o_pair)
                        # pops go between S and the lagged O pair: the PE
                        # chews background work while the previous slab's
                        # exp finishes, instead of stalling on psl
                        for _ in range(per):
                            if pend:
                                pend.pop(0)()
                        horizon -= 1
                        if mt >= 1:
                            o_emits.pop(0)()
                    while o_emits:
                        o_emits.pop(0)()
                    # flush O_aug psums into the chunk accumulator
                    if blk == 0:
                        nc.vector.tensor_copy(acc[c][:, 2 * hp, :], ops_e)
                        nc.vector.tensor_copy(acc[c][:, 2 * hp + 1, :], ops_o)
                    else:
                        nc.vector.tensor_add(
                            acc[c][:, 2 * hp, :], acc[c][:, 2 * hp, :], ops_e
                        )
                        nc.vector.tensor_add(
                            acc[c][:, 2 * hp + 1, :], acc[c][:, 2 * hp + 1, :],
                            ops_o,
                        )
                    if blk == NBLK - 1:
                        pend = pend + make_p3_hp_thunks(c, hp, part="head")
                        if hp > 0:
                            pend = pend + make_p3_hp_thunks(
                                c, hp - 1, part="tail"
                            )
            if blk == NBLK - 1:
                pend = pend + make_p3_hp_thunks(c, HP - 1, part="tail")
            return pend, horizon

        p1_pass((0, 1), [])
        for t in p1_pass((2, 3), make_proj_thunks(0)):
            t()
        # interleave (chunk, block) groups so block b+1's projection
        # spreads over BOTH groups that process block b (32 slabs),
        # keeping every slab PE-bound instead of exp-bound; (0,7) comes
        # before (1,6)/(1,7) so chunk 0's P3/P4 hide under them.
        groups = [(0, 0)]
        for b in range(6):
            groups += [(1, b), (0, b + 1)]
        groups += [(0, 7), (1, 6), (1, 7)]
        pend, horizon = [], 16
        for c, blk in groups:
            if c == 0 and blk + 1 < NBLK:
                pend = pend + make_proj_thunks(blk + 1)
                horizon = 16 if blk == 6 else 32
            if (c, blk) == (1, 6):
                pend = pend + make_p4_thunks(0)
                horizon = 32
            horizon = max(horizon, 16) if pend else 16
            pend, horizon = run_block(c, blk, pend, horizon)
        # tail: chunk 1's last P3 chain + P4(1).  Per-group thunks are
        # [alloc, mm hp0..3, ev] x2 + dma (12 each).  Emit the recip
        # DMAs, then two groups' alloc+hp0-2 matmuls (their psums plus
        # the two bcast psums exactly fill the o_ps ring) so the PE
        # stays busy while the chain flies, then the chain, then the
        # gated hp3 matmuls and the remaining groups.
        p4 = make_p4_thunks(1)
        g0, g1, rest = p4[0:12], p4[12:24], p4[24:]
        # pend's last 5 thunks are always hp3's P3 tail (appended last,
        # FIFO pops take from the front); everything before it must run
        # before P4's hp0-2 matmuls (ko writers).  Hoist two groups'
        # alloc+hp0-2 matmuls over the hp3 tail so the PE stays busy
        # while its bcast waits on the recip-chain DMAs.
        tail = (
            pend[:-5]
            + g0[0:4] + g1[0:4]          # allocs + mm hp0-2 of nck 0
            + pend[-5:]                   # hp3 bcast/mul/repack
            + g0[4:] + g1[4:]            # mm hp3, ev, second nck, dma
            + rest
        )
        for t in tail:
            t()


_NC_CACHE = None


def _get_nc():
    global _NC_CACHE
    if _NC_CACHE is None:
        _NC_CACHE = build_nc()
    return _NC_CACHE


def shard_inputs(x, context, Wq, Wk, Wv, Wo, bo):
    import ml_dtypes

    bf16 = ml_dtypes.bfloat16
    ones = np.ones((1, 128), np.float32).astype(bf16)
    bo2 = np.ascontiguousarray(np.asarray(bo, np.float32).reshape(1, DQ))
    Wq = np.ascontiguousarray(np.asarray(Wq, np.float32).astype(bf16))
    Wk = np.ascontiguousarray(np.asarray(Wk, np.float32).astype(bf16))
    Wv = np.ascontiguousarray(np.asarray(Wv, np.float32).astype(bf16))
    Wo = np.ascontiguousarray(np.asarray(Wo, np.float32).astype(bf16))
    maps = []
    for c in range(NCORES):
        b = c // 4
        r0 = (c % 4) * NQ
        maps.append(
            {
                "xT": np.ascontiguousarray(x[b, r0 : r0 + NQ, :].T.astype(bf16)),
                "ctxT": np.ascontiguousarray(context[b].T.astype(bf16)),
                "wq": Wq,
                "wk": Wk,
                "wv": Wv,
                "wo": Wo,
                "bo": bo2,
                "ones_d": ones,
            }
        )
    return maps


def kernel(x, context, Wq, Wk, Wv, Wo, bo):
    from concourse.bass_utils import run_bass_kernel_spmd

    x = np.asarray(x, np.float32)
    context = np.asarray(context, np.float32)
    maps = shard_inputs(x, context, Wq, Wk, Wv, Wo, bo)
    nc = _get_nc()
    trace = os.environ.get("KERNEL_TRACE", "0") == "1"
    res = run_bass_kernel_spmd(
        nc, maps, core_ids=list(range(NCORES)), trace=trace
    )
    full = np.empty((B, N, DQ), np.float32)
    for c in range(NCORES):
        b = c // 4
        r0 = (c % 4) * NQ
        full[b, r0 : r0 + NQ, :] = res.results[c]["out"]
    if trace:
        kernel.last_exec_time_ns = res.exec_time_ns
    return full
